# revision 8
# baseline (speedup 1.0000x reference)
"""CrossAttention (DFFNet) Trainium2 Bass kernel.

Shapes (hardcoded): rgb/depth [4, 256, 64, 64] f32; Wq/Wk [32, 256]; Wv [256, 256].

    q = Wq @ d + bq          [B, 32, 4096]
    k = Wk @ d + bk          [B, 32, 4096]
    v = Wv @ r + bv          [B, 256, 4096]
    scores = q^T k           [B, 4096, 4096], softmax over keys (last dim)
    feat = v @ mask^T        [B, 256, 4096]

Sharding: 8 cores = 4 batches x 2 query-halves (2048 queries each). Each core
gets full depth/rgb for its batch (keys/values span all 4096 tokens) plus its
query-half slice of depth.

Device layout choice: scores are computed TRANSPOSED, st[m, n] (keys m on
partitions, queries n free) so the feat matmul needs no transposes:
  - v^T[m, c] is produced directly by  r-slice^T @ Wv^T  (r already has
    channels on partitions, which is the contraction dim).
  - feat[c, n] = sum_m v^T[m, c] * exp(st[m, n]) / S[n]  -> lhsT = v^T tile,
    rhs = exp(st) tile, both with m on partitions.
  - softmax denominator S[n] = sum_m exp(st[m,n]) is a partition-axis sum ->
    ones[128,1]-lhsT matmul accumulated over m-tiles (PE streams it).
  - no max-subtraction: |scores| < ~6 here, exp is well-conditioned.
Normalization: 1/S via fast reciprocal (DVE), broadcast to 128 partitions via
a K=1 matmul with a ones row, multiply + bias-add on DVE.

The K=32 score matmuls are 4-way row-packed (tile_position=(32j, 0)): k and q
are kept in 4x-replicated layouts [128, *] (4 copies at partition offsets
0/32/64/96), which fall out of the projection matmuls for free by tiling the
tiny weight matrices host-side (WkT_4x = tile(Wk.T, (1, 4))).

All matmuls run as float32r (full PE rate at free-dim >= 256, fp32 storage).
"""

import numpy as np

import concourse.bacc as bacc
import concourse.bass as bass
import concourse.mybir as mybir
import concourse.tile as tile
from concourse.bass_utils import run_bass_kernel_spmd

B, C, H, W = 4, 256, 64, 64
HW = H * W            # 4096
CQK = 32
P = 128
NQ = HW // 2          # 2048 queries per core
NT = 512              # query tile
N_NT = NQ // NT       # 4
MT = HW // P          # 32 key tiles
KC = C // P           # 2 contraction tiles for the projections

F32 = mybir.dt.float32
F32R = mybir.dt.float32r
BF16 = mybir.dt.bfloat16
AF = mybir.ActivationFunctionType
OP = mybir.AluOpType


def _r(ap):
    """View an fp32 AP as float32r (valid only after _round_inplace)."""
    return ap.bitcast(F32R)


def _staged_load(nc, pool, dst, dram_ap, chunk=1024):
    """DMA fp32 DRAM -> small staging tile, DVE-copy (rounding) -> f32r dst."""
    n = dst.shape[1]
    for c0 in range(0, n, chunk):
        w = min(chunk, n - c0)
        stg = pool.tile([P, chunk], F32, tag="stage", name=f"stg_{dst.name}_{c0}")
        nc.sync.dma_start(stg[:, 0:w], dram_ap[:, c0:c0 + w])
        nc.vector.tensor_copy(dst[:, c0:c0 + w], stg[:, 0:w])


def _emit(tc, io):
    nc = tc.nc
    d = io["d"].ap()          # [256, 4096] depth (keys source)
    dq = io["dq"].ap()        # [256, 2048] depth query-half
    r = io["r"].ap()          # [256, 4096] rgb (values source)
    wqt4 = io["wqt4"].ap()    # [256, 128] = tile(Wq.T, (1,4))
    wkt4 = io["wkt4"].ap()    # [256, 128]
    wvt = io["wvt"].ap()      # [256, 256] = Wv.T
    bq4 = io["bq4"].ap()      # [128, 1] = tile(bq, 4)
    bk4 = io["bk4"].ap()      # [128, 1]
    bv2 = io["bv2"].ap()      # [256, 1]
    out = io["out"].ap()      # [256, 2048]

    from contextlib import ExitStack

    with ExitStack() as ctx:
        pw = ctx.enter_context(tc.tile_pool(name="weights", bufs=1))
        pin = ctx.enter_context(tc.tile_pool(name="inputs", bufs=1))
        pqk = ctx.enter_context(tc.tile_pool(name="qk", bufs=1))
        pvt = ctx.enter_context(tc.tile_pool(name="vt", bufs=1))
        pse = ctx.enter_context(tc.tile_pool(name="stexp", bufs=2))
        psmall = ctx.enter_context(tc.tile_pool(name="small", bufs=2))
        pout = ctx.enter_context(tc.tile_pool(name="outsb", bufs=4))
        pstage = ctx.enter_context(tc.tile_pool(name="stage", bufs=4))
        ps_st = ctx.enter_context(
            tc.tile_pool(name="ps_st", bufs=2, space=bass.MemorySpace.PSUM))
        ps_feat = ctx.enter_context(
            tc.tile_pool(name="ps_feat", bufs=2, space=bass.MemorySpace.PSUM))
        ps_sums = ctx.enter_context(
            tc.tile_pool(name="ps_sums", bufs=1, space=bass.MemorySpace.PSUM))
        ps_bc = ctx.enter_context(
            tc.tile_pool(name="ps_bc", bufs=1, space=bass.MemorySpace.PSUM))

        # ---- constants / weights --------------------------------------
        wq_t, wk_t, wv_t, bv_t = [], [], [], []
        for kc in range(KC):
            t = pw.tile([P, P], BF16, tag=f"wq{kc}")
            _staged_load(nc, pstage, t, wqt4[kc * P:(kc + 1) * P, :])
            wq_t.append(t)
            t = pw.tile([P, P], BF16, tag=f"wk{kc}")
            _staged_load(nc, pstage, t, wkt4[kc * P:(kc + 1) * P, :])
            wk_t.append(t)
            t = pw.tile([P, C], BF16, tag=f"wv{kc}")
            _staged_load(nc, pstage, t, wvt[kc * P:(kc + 1) * P, :])
            wv_t.append(t)
            t = pw.tile([P, 1], F32, tag=f"bv{kc}")
            nc.sync.dma_start(t[:], bv2[kc * P:(kc + 1) * P, :])
            bv_t.append(t)
        bq_sb = pw.tile([P, 1], F32, tag="bq")
        nc.sync.dma_start(bq_sb[:], bq4[:])
        bk_sb = pw.tile([P, 1], F32, tag="bk")
        nc.sync.dma_start(bk_sb[:], bk4[:])
        ones_f = pw.tile([P, 1], F32, tag="ones_f")
        nc.vector.memset(ones_f[:], 1.0)
        ones_col = pw.tile([P, 1], BF16, tag="ones_col")
        nc.vector.tensor_copy(ones_col[:], ones_f[:])
        ones_row = pw.tile([1, P], F32, tag="ones_row")
        nc.vector.memset(ones_row[:], 1.0)

        # ---- inputs ----------------------------------------------------
        d_sb, dq_sb, r_sb = [], [], []
        for kc in range(KC):
            t = pin.tile([P, HW], BF16, tag=f"d{kc}")
            _staged_load(nc, pstage, t, d[kc * P:(kc + 1) * P, :])
            d_sb.append(t)
            t = pin.tile([P, NQ], BF16, tag=f"dq{kc}")
            _staged_load(nc, pstage, t, dq[kc * P:(kc + 1) * P, :])
            dq_sb.append(t)
            t = pin.tile([P, HW], BF16, tag=f"r{kc}")
            _staged_load(nc, pstage, t, r[kc * P:(kc + 1) * P, :])
            r_sb.append(t)

        # ---- k / q projections (4x-replicated layouts) -----------------
        # k4[32j + o, m] = k[o, m];  q4[32j + o, n] = q[o, n]
        k4 = pqk.tile([P, HW], BF16, tag="k4")
        for qtr in range(4):
            kp = ps_st.tile([P, 1024], F32, tag="stp", name=f"kp{qtr}")
            for sub in range(2):
                n0 = sub * NT
                g0 = qtr * 1024 + n0
                for kc in range(KC):
                    nc.tensor.matmul(
                        kp[:, n0:n0 + NT],
                        lhsT=wk_t[kc][:],
                        rhs=d_sb[kc][:, g0:g0 + NT],
                        start=(kc == 0),
                        stop=(kc == KC - 1),
                    )
            nc.vector.tensor_scalar(
                k4[:, qtr * 1024:(qtr + 1) * 1024], kp[:], bk_sb[:], None, OP.add
            )
        q4 = pqk.tile([P, NQ], BF16, tag="q4")
        for half in range(2):
            qp = ps_st.tile([P, 1024], F32, tag="stp", name=f"qp{half}")
            for sub in range(2):
                n0 = sub * NT
                g0 = half * 1024 + n0
                for kc in range(KC):
                    nc.tensor.matmul(
                        qp[:, n0:n0 + NT],
                        lhsT=wq_t[kc][:],
                        rhs=dq_sb[kc][:, g0:g0 + NT],
                        start=(kc == 0),
                        stop=(kc == KC - 1),
                    )
            nc.vector.tensor_scalar(
                q4[:, half * 1024:(half + 1) * 1024], qp[:], bq_sb[:], None, OP.add
            )

        # ---- v^T projection: vt[mt][p, c] = v[c, mt*128 + p] (no bias) --
        vt_t = []
        for mt in range(MT):
            vp = ps_feat.tile([P, C], F32, tag="feat")
            for kc in range(KC):
                nc.tensor.matmul(
                    vp[:],
                    lhsT=r_sb[kc][:, mt * P:(mt + 1) * P],
                    rhs=wv_t[kc][:],
                    start=(kc == 0),
                    stop=(kc == KC - 1),
                )
            t = pvt.tile([P, C], BF16, tag=f"vt{mt}")
            nc.vector.tensor_copy(t[:], vp[:])
            vt_t.append(t)

        # ---- main attention loop ---------------------------------------
        for nt in range(N_NT):
            n0 = nt * NT
            fc = [ps_feat.tile([P, NT], F32, tag="feat", name=f"fc{nt}_{i}") for i in range(2)]
            sm = ps_sums.tile([1, NT], F32, tag="sums")
            for g in range(MT // 2):
                stp = ps_st.tile([P, 1024], F32, tag="stp", name=f"stp{nt}_{g}")
                for j in range(2):
                    mt = 2 * g + j
                    nc.tensor.matmul(
                        stp[:, j * NT:(j + 1) * NT],
                        lhsT=k4[32 * j:32 * j + 32, mt * P:(mt + 1) * P],
                        rhs=q4[32 * j:32 * j + 32, n0:n0 + NT],
                        start=True,
                        stop=True,
                        tile_position=(32 * j, 0),
                    )
                se = pse.tile([P, 1024], BF16, tag="se", name=f"se{nt}_{g}")
                nc.scalar.activation(se[:], stp[:], AF.Exp)
                for j in range(2):
                    mt = 2 * g + j
                    sej = se[:, j * NT:(j + 1) * NT]
                    first = mt == 0
                    last = mt == MT - 1
                    nc.tensor.matmul(
                        fc[0][:], lhsT=vt_t[mt][:, 0:P], rhs=sej,
                        start=first, stop=last,
                    )
                    nc.tensor.matmul(
                        fc[1][:], lhsT=vt_t[mt][:, P:C], rhs=sej,
                        start=first, stop=last,
                    )
                    nc.tensor.matmul(
                        sm[:], lhsT=ones_col[:], rhs=sej,
                        start=first, stop=last,
                    )
            rc = psmall.tile([1, NT], F32, tag="recip")
            nc.vector.reciprocal_approx_fast(out=rc[:], in_=sm[:])
            bc = ps_bc.tile([P, NT], F32, tag="bc")
            nc.tensor.matmul(
                bc[:], lhsT=ones_row[:], rhs=rc[:], start=True, stop=True
            )
            bc_sb = pout.tile([P, NT], F32, tag="bc_sb")
            nc.vector.tensor_copy(bc_sb[:], bc[:])
            for c in range(2):
                tmp = pout.tile([P, NT], F32, tag="tmp")
                nc.vector.tensor_tensor(tmp[:], fc[c][:], bc_sb[:], OP.mult)
                ot = pout.tile([P, NT], F32, tag="ot")
                nc.vector.tensor_scalar(ot[:], tmp[:], bv_t[c][:], None, OP.add)
                nc.sync.dma_start(out[c * P:(c + 1) * P, n0:n0 + NT], ot[:])


_BUILT = None


def _build():
    global _BUILT
    if _BUILT is not None:
        return _BUILT
    nc = bacc.Bacc("TRN2", target_bir_lowering=False, debug=False)
    io = {
        "d": nc.dram_tensor("d", [C, HW], F32, kind="ExternalInput"),
        "dq": nc.dram_tensor("dq", [C, NQ], F32, kind="ExternalInput"),
        "r": nc.dram_tensor("r", [C, HW], F32, kind="ExternalInput"),
        "wqt4": nc.dram_tensor("wqt4", [C, P], F32, kind="ExternalInput"),
        "wkt4": nc.dram_tensor("wkt4", [C, P], F32, kind="ExternalInput"),
        "wvt": nc.dram_tensor("wvt", [C, C], F32, kind="ExternalInput"),
        "bq4": nc.dram_tensor("bq4", [P, 1], F32, kind="ExternalInput"),
        "bk4": nc.dram_tensor("bk4", [P, 1], F32, kind="ExternalInput"),
        "bv2": nc.dram_tensor("bv2", [C, 1], F32, kind="ExternalInput"),
        "out": nc.dram_tensor("out", [C, NQ], F32, kind="ExternalOutput"),
    }
    with tile.TileContext(nc) as tc:
        _emit(tc, io)
    nc.compile()
    _BUILT = nc
    return nc


def _in_maps(rgb, depth, Wq, bq, Wk, bk, Wv, bv):
    f = np.float32
    d_all = np.ascontiguousarray(depth.reshape(B, C, HW), dtype=f)
    r_all = np.ascontiguousarray(rgb.reshape(B, C, HW), dtype=f)
    wqt4 = np.ascontiguousarray(np.tile(np.asarray(Wq, f).T, (1, 4)))
    wkt4 = np.ascontiguousarray(np.tile(np.asarray(Wk, f).T, (1, 4)))
    wvt = np.ascontiguousarray(np.asarray(Wv, f).T)
    bq4 = np.ascontiguousarray(np.tile(np.asarray(bq, f), 4).reshape(P, 1))
    bk4 = np.ascontiguousarray(np.tile(np.asarray(bk, f), 4).reshape(P, 1))
    bv2 = np.ascontiguousarray(np.asarray(bv, f).reshape(C, 1))
    maps = []
    for core in range(8):
        b, half = core // 2, core % 2
        maps.append({
            "d": d_all[b],
            "dq": np.ascontiguousarray(d_all[b][:, half * NQ:(half + 1) * NQ]),
            "r": r_all[b],
            "wqt4": wqt4, "wkt4": wkt4, "wvt": wvt,
            "bq4": bq4, "bk4": bk4, "bv2": bv2,
        })
    return maps


def kernel(rgb, depth, Wq, bq, Wk, bk, Wv, bv, **run_kwargs):
    nc = _build()
    maps = _in_maps(rgb, depth, Wq, bq, Wk, bk, Wv, bv)
    res = run_bass_kernel_spmd(nc, maps, core_ids=list(range(8)), **run_kwargs)
    results = res.results if hasattr(res, "results") else res
    out = np.empty((B, C, HW), dtype=np.float32)
    for core in range(8):
        b, half = core // 2, core % 2
        out[b][:, half * NQ:(half + 1) * NQ] = results[core]["out"]
    kernel.last_results = res
    return out.reshape(B, C, H, W)
